# revision 9
# baseline (speedup 1.0000x reference)
"""Trainium2 Bass kernel for the DLDetFPNHead embedding-loss reduction.

Contract: kernel(**inputs) takes FULL unsharded inputs
  pred_emb   (2, 80, 512, 512)  float32
  gt_objmask (2, 32, 512, 512)  bool
  gt_classes (2, 32)            int
and returns the FULL output: a float32 scalar (shape ()).

Strategy (hardcoded): the 64 (n, k) instances are sharded 8-per-core across
8 NeuronCores.  Host-side sharding gathers, for each instance, its class
channel pred_emb[n, gt_classes[n, k]] (fp16) and its mask (fp16).  Each core
computes per-instance masked sum / sum-of-squares / count over the 512x512
map; the host finishes with the tiny O(K^2) pairwise term.
"""

import os
import numpy as np
from contextlib import ExitStack

import concourse.bacc as bacc
import concourse.mybir as mybir
import concourse.tile as tile
from concourse import bass
from concourse.bass_utils import run_bass_kernel_spmd

N_CORES = 8
NI = 8            # instances per core  (2 images * 32 instances / 8 cores)
P = 128           # SBUF partitions
FD = 2048         # 512*512 / 128 free elements per partition
DT = mybir.dt.float16

TRACE = False     # test harness can flip this to get a profiled run
LAST_BASS_RESULTS = None


def _install_ntff_shim():
    """This container's antenv lacks axon_hooks; synthesize it so
    run_bass_kernel_spmd(trace=True) can reach the axon NTFF profiler."""
    import sys, types

    try:
        from antenv.axon_hooks import get_axon_ntff_profile_hook  # noqa: F401

        return
    except ImportError:
        pass
    try:
        import antenv
        from trn_agent_boot.trn_boot import _ntff_profile_via_ctypes

        mod = types.ModuleType("antenv.axon_hooks")
        _h = [None]
        mod.set_axon_ntff_profile_hook = lambda h: _h.__setitem__(0, h)
        mod.get_axon_ntff_profile_hook = lambda: _h[0]
        sys.modules["antenv.axon_hooks"] = mod
        antenv.axon_hooks = mod
        mod.set_axon_ntff_profile_hook(
            _ntff_profile_via_ctypes("/opt/axon/libaxon_pjrt.so")
        )
    except Exception as e:  # profiling is best-effort; execution still works
        print(f"ntff shim failed: {e}")


def build_nc():
    nc = bacc.Bacc("TRN2", target_bir_lowering=False, debug=False)

    NCH = FD // 512  # 512-column chunks per instance for PE column sums

    emb_d = nc.dram_tensor("emb", [NI, P, FD], DT, kind="ExternalInput")
    msk_d = nc.dram_tensor("msk", [NI, P, FD], DT, kind="ExternalInput")
    out_s_d = nc.dram_tensor("out_s", [1, NI], mybir.dt.float32, kind="ExternalOutput")
    out_s2_d = nc.dram_tensor("out_s2", [1, NI], mybir.dt.float32, kind="ExternalOutput")
    out_c_d = nc.dram_tensor("out_cnt", [NI, 1], mybir.dt.float32, kind="ExternalOutput")

    with tile.TileContext(nc) as tc, ExitStack() as ctx:
        epool = ctx.enter_context(tc.tile_pool(name="e", bufs=3))
        mpool = ctx.enter_context(tc.tile_pool(name="m", bufs=3))
        empool = ctx.enter_context(tc.tile_pool(name="em", bufs=3))
        scratch = ctx.enter_context(tc.tile_pool(name="scratch", bufs=2))
        stats = ctx.enter_context(tc.tile_pool(name="stats", bufs=1))
        psum = ctx.enter_context(
            tc.tile_pool(name="ps", bufs=1, space=bass.MemorySpace.PSUM)
        )

        s_part = stats.tile([P, NI], mybir.dt.float32)
        s2_part = stats.tile([P, NI], mybir.dt.float32)
        ones32 = stats.tile([P, 1], mybir.dt.float32)
        nc.gpsimd.memset(ones32[:], 1.0)
        # sel[:, i*NI + j] = (i == j): per-instance all-ones selector column,
        # so instance i's matmul lands on PSUM row i (base partition must be 0)
        sel = stats.tile([P, NI * NI], DT)
        nc.gpsimd.memset(sel[:], 0.0)
        for i in range(NI):
            nc.gpsimd.memset(sel[:, i * NI + i : i * NI + i + 1], 1.0)

        psum_c = psum.tile([NI, 512], mybir.dt.float32, tag="ps_c")

        for i in range(NI):
            e = epool.tile([P, FD], DT)
            m = mpool.tile([P, FD], DT)
            nc.sync.dma_start(e[:], emb_d[i])
            nc.sync.dma_start(m[:], msk_d[i])

            # em = e * m ; s_part[:, i] = sum_fd(em)  (one fused DVE op)
            em = empool.tile([P, FD], DT)
            nc.vector.affine_mul_reduce(
                out=em[:],
                accum_out=s_part[:, i : i + 1],
                in0=e[:],
                in1=m[:],
                scale=1.0,
                bias=0.0,
            )
            # s2_part[:, i] = sum_fd(em^2)   (ACT square with accumulate)
            sq = scratch.tile([P, FD], DT, tag="sq")
            nc.scalar.activation(
                sq[:],
                em[:],
                mybir.ActivationFunctionType.Square,
                accum_out=s2_part[:, i : i + 1],
            )
            # cnt via PE: psum_c[i, :] += sel_i^T @ m_chunk (rows != i add zero)
            for c in range(NCH):
                nc.tensor.matmul(
                    psum_c[:, :],
                    sel[:, i * NI : (i + 1) * NI],
                    m[:, 512 * c : 512 * (c + 1)],
                    start=(i == 0 and c == 0),
                    stop=(i == NI - 1 and c == NCH - 1),
                )

        # finish: cnt rows (NI,512)->(NI,1); s/s2 cross-partition via PE
        c_col = stats.tile([NI, 1], mybir.dt.float32)
        nc.vector.tensor_reduce(
            c_col[:], psum_c[:], axis=mybir.AxisListType.X, op=mybir.AluOpType.add
        )
        nc.sync.dma_start(out_c_d[:], c_col[:])

        acc = psum.tile([1, 2 * NI], mybir.dt.float32, tag="ps_fin")
        nc.tensor.matmul(acc[:, 0:NI], ones32[:], s_part[:], start=True, stop=True)
        nc.tensor.matmul(acc[:, NI:], ones32[:], s2_part[:], start=True, stop=True)
        res = stats.tile([1, 2 * NI], mybir.dt.float32)
        nc.vector.tensor_copy(res[:], acc[:])
        nc.sync.dma_start(out_s_d[:], res[0:1, 0:NI])
        nc.sync.dma_start(out_s2_d[:], res[0:1, NI:])

    nc.compile()
    return nc


_NC = None


def _get_nc():
    global _NC
    if _NC is None:
        _NC = build_nc()
    return _NC


def _device_stats(emb_sel, msk):
    """emb_sel, msk: (64, P, FD) float16 -> s, s2, cnt each (64,) float32."""
    global LAST_BASS_RESULTS
    if TRACE:
        _install_ntff_shim()
    nc = _get_nc()
    in_maps = [
        {
            "emb": np.ascontiguousarray(emb_sel[c * NI : (c + 1) * NI]),
            "msk": np.ascontiguousarray(msk[c * NI : (c + 1) * NI]),
        }
        for c in range(N_CORES)
    ]
    br = run_bass_kernel_spmd(
        nc,
        in_maps,
        list(range(N_CORES)),
        trace=TRACE,
        trace_cores=list(range(N_CORES)) if TRACE else None,
    )
    LAST_BASS_RESULTS = br
    s = np.concatenate([br.results[c]["out_s"][0] for c in range(N_CORES)])
    s2 = np.concatenate([br.results[c]["out_s2"][0] for c in range(N_CORES)])
    cnt = np.concatenate([br.results[c]["out_cnt"][:, 0] for c in range(N_CORES)])
    return s, s2, cnt


def kernel(pred_emb, gt_objmask, gt_classes):
    pred_emb = np.asarray(pred_emb)
    mask = np.asarray(gt_objmask)
    cls = np.asarray(gt_classes)
    N, C, H, W = pred_emb.shape
    K = cls.shape[1]
    assert (N * K, H * W) == (N_CORES * NI, P * FD)

    # host-side sharding: gather each instance's class channel, fp16 wire format
    emb_sel = pred_emb[np.arange(N)[:, None], cls]          # (N, K, H, W) f32
    emb_sel = emb_sel.reshape(N * K, P, FD).astype(np.float16)
    msk = mask.reshape(N * K, P, FD).astype(np.float16)

    s, s2, cnt = _device_stats(emb_sel, msk)
    s = s.astype(np.float64).reshape(N, K)
    s2 = s2.astype(np.float64).reshape(N, K)
    cnt = cnt.astype(np.float64).reshape(N, K)

    # tiny epilogue (exact mirror of the reference math)
    valid = cnt > 0
    safe = np.where(valid, cnt, 1.0)
    mean = np.where(valid, s / safe, 0.0)
    var = np.where(valid, s2 / safe - mean * mean, 0.0)
    per_image = np.zeros(N)
    triu = np.triu(np.ones((K, K), dtype=bool), k=1)
    for n in range(N):
        d2 = (mean[n][:, None] - mean[n][None, :]) ** 2
        same = cls[n][:, None] == cls[n][None, :]
        inter = np.sum(np.where(same & triu, np.maximum(1.0 - d2, 0.0), 0.0))
        per_image[n] = inter + np.mean(mean[n] ** 2) + np.mean(var[n])
    return np.float32(per_image.mean() * 0.1)
